# revision 5
# baseline (speedup 1.0000x reference)
"""Distributed attention kernel for Trainium2 (8 NeuronCores, SPMD).

Problem: B=4, S=4096, D=256 attention with QKV linear projections.
Sharding: core = 2*b + half -> batch b, query rows [half*2048, (half+1)*2048),
full K/V for that batch on every core (data-parallel batch + seq-parallel q).

Device-side layout trick: all sequence tensors are pre-transposed on the host
to [D, seq] so every matmul contraction dim lands on the partition axis and no
on-device transposes are needed. Scores are computed transposed
(scoresT[j, i] = kp . qp) so softmax exp is purely elementwise and the
AV matmul consumes exp tiles directly as the stationary operand.
Row-sums are obtained by appending a ones-column to vp (rank-1 matmul also
folds in the v-bias), so normalization is a cheap reciprocal+scale at the end.
"""

import math
from contextlib import ExitStack

import numpy as np
import ml_dtypes

B = 4
S = 4096
D = 256
SQ = 2048  # query rows per core
NE = 2  # number of 128-row tiles covering D
ICH = 512  # i-chunk (query columns per scoresT matmul)
NIC = SQ // ICH  # 4
NJ = S // 128  # 32 j-tiles
H1 = D + 1  # vp width including the ones column

_CACHE = {}
_last_in_maps = None


def _build():
    import concourse.bass as bass
    import concourse.tile as tile
    from concourse import bacc, mybir

    BF = mybir.dt.bfloat16
    F32 = mybir.dt.float32
    AF = mybir.ActivationFunctionType

    nc = bacc.Bacc("TRN2", target_bir_lowering=False, debug=False)

    qT = nc.declare_dram_parameter("qT", [D, SQ], BF, isOutput=False)
    kT = nc.declare_dram_parameter("kT", [D, S], BF, isOutput=False)
    vT = nc.declare_dram_parameter("vT", [D, S], BF, isOutput=False)
    WqT = nc.declare_dram_parameter("WqT", [D, D], BF, isOutput=False)
    WkT = nc.declare_dram_parameter("WkT", [D, D], BF, isOutput=False)
    WvT = nc.declare_dram_parameter("WvT", [D, D], BF, isOutput=False)
    bqs = nc.declare_dram_parameter("bqs", [D, 1], F32, isOutput=False)
    bk = nc.declare_dram_parameter("bk", [D, 1], F32, isOutput=False)
    bv1 = nc.declare_dram_parameter("bv1", [1, H1], BF, isOutput=False)
    out = nc.declare_dram_parameter("out", [SQ, D], F32, isOutput=True)

    with TileKernel(nc, tile) as (tc, ctx):
        const = ctx.enter_context(tc.tile_pool(name="const", bufs=1))
        inp = ctx.enter_context(tc.tile_pool(name="inp", bufs=1))
        persist = ctx.enter_context(tc.tile_pool(name="persist", bufs=1))

        # --- load weights/biases/constants ---
        wq_sb = []
        wk_sb = []
        wv_sb = []
        for dt in range(NE):
            t = const.tile([128, D], BF, tag=f"wq{dt}")
            nc.sync.dma_start(t[:], WqT[dt * 128:(dt + 1) * 128, :])
            wq_sb.append(t)
            t = const.tile([128, D], BF, tag=f"wk{dt}")
            nc.sync.dma_start(t[:], WkT[dt * 128:(dt + 1) * 128, :])
            wk_sb.append(t)
            t = const.tile([128, D], BF, tag=f"wv{dt}")
            nc.sync.dma_start(t[:], WvT[dt * 128:(dt + 1) * 128, :])
            wv_sb.append(t)
        bqs_sb = []
        bk_sb = []
        for et in range(NE):
            t = const.tile([128, 1], F32, tag=f"bqs{et}")
            nc.sync.dma_start(t[:], bqs[et * 128:(et + 1) * 128, :])
            bqs_sb.append(t)
            t = const.tile([128, 1], F32, tag=f"bk{et}")
            nc.sync.dma_start(t[:], bk[et * 128:(et + 1) * 128, :])
            bk_sb.append(t)
        bv1_sb = const.tile([1, H1], BF, tag="bv1")
        nc.sync.dma_start(bv1_sb[:], bv1[:])
        ones1 = const.tile([1, 128], BF, tag="ones1")
        nc.vector.memset(ones1[:], 1.0)

        # --- load inputs (pre-transposed on host) ---
        qT_sb = []
        kT_sb = []
        vT_sb = []
        for dt in range(NE):
            t = inp.tile([128, SQ], BF, tag=f"qT{dt}")
            nc.sync.dma_start(t[:], qT[dt * 128:(dt + 1) * 128, :])
            qT_sb.append(t)
            t = inp.tile([128, S], BF, tag=f"kT{dt}")
            nc.sync.dma_start(t[:], kT[dt * 128:(dt + 1) * 128, :])
            kT_sb.append(t)
            t = inp.tile([128, S], BF, tag=f"vT{dt}")
            nc.sync.dma_start(t[:], vT[dt * 128:(dt + 1) * 128, :])
            vT_sb.append(t)

        # --- projections ---
        qpT_sb = [persist.tile([128, SQ], BF, tag=f"qpT{et}", name=f"qpT{et}") for et in range(NE)]
        kpT_sb = [persist.tile([128, S], BF, tag=f"kpT{et}", name=f"kpT{et}") for et in range(NE)]
        vp_sb = [persist.tile([128, H1], BF, tag=f"vp{j}", name=f"vp{j}") for j in range(NJ)]

        with tc.tile_pool(name="ppsum", bufs=2, space=bass.MemorySpace.PSUM) as ppsum, \
             tc.tile_pool(name="vpsum", bufs=2, space=bass.MemorySpace.PSUM) as vpsum:
            # qpT[e, i] = sum_d WqT_s[d, e] * qT[d, i]  (+ s*bq)
            for et in range(NE):
                esl = slice(et * 128, (et + 1) * 128)
                for c in range(SQ // 512):
                    csl = slice(c * 512, (c + 1) * 512)
                    pp = ppsum.tile([128, 512], F32, tag="pp")
                    nc.tensor.matmul(pp[:], wq_sb[0][:, esl], qT_sb[0][:, csl],
                                     start=True, stop=False)
                    nc.tensor.matmul(pp[:], wq_sb[1][:, esl], qT_sb[1][:, csl],
                                     start=False, stop=True)
                    nc.vector.tensor_scalar_add(qpT_sb[et][:, csl], pp[:], bqs_sb[et][:])
            # kpT[e, j] = sum_d WkT[d, e] * kT[d, j]  (+ bk)
            for et in range(NE):
                esl = slice(et * 128, (et + 1) * 128)
                for c in range(S // 512):
                    csl = slice(c * 512, (c + 1) * 512)
                    pp = ppsum.tile([128, 512], F32, tag="pp")
                    nc.tensor.matmul(pp[:], wk_sb[0][:, esl], kT_sb[0][:, csl],
                                     start=True, stop=False)
                    nc.tensor.matmul(pp[:], wk_sb[1][:, esl], kT_sb[1][:, csl],
                                     start=False, stop=True)
                    nc.vector.tensor_scalar_add(kpT_sb[et][:, csl], pp[:], bk_sb[et][:])
            # vp[j, h] = sum_d vT[d, j] * WvT[d, h] + bv[h]; col D is all-ones
            for j in range(NJ):
                jsl = slice(j * 128, (j + 1) * 128)
                pv = vpsum.tile([128, H1], F32, tag="pv")
                nc.tensor.matmul(pv[:, 0:D], vT_sb[0][:, jsl], wv_sb[0][:],
                                 start=True, stop=False, skip_group_check=True)
                nc.tensor.matmul(pv[:, 0:D], vT_sb[1][:, jsl], wv_sb[1][:],
                                 start=False, stop=False, skip_group_check=True)
                nc.tensor.matmul(pv[:, 0:H1], ones1[:], bv1_sb[:],
                                 start=False, stop=True, skip_group_check=True)
                nc.scalar.activation(vp_sb[j][:], pv[:], AF.Identity)

        # --- attention ---
        with tc.tile_pool(name="spsum", bufs=4, space=bass.MemorySpace.PSUM) as spsum, \
             tc.tile_pool(name="opsum", bufs=1, space=bass.MemorySpace.PSUM) as opsum, \
             tc.tile_pool(name="expp", bufs=4) as expp, \
             tc.tile_pool(name="norm", bufs=4) as norm, \
             tc.tile_pool(name="obuf", bufs=4) as obuf:
            for ic in range(NIC):
                icsl = slice(ic * ICH, (ic + 1) * ICH)
                otiles = [opsum.tile([128, H1], F32, tag=f"ot{it}", name=f"ot{ic}_{it}") for it in range(4)]
                for j in range(NJ):
                    jsl = slice(j * 128, (j + 1) * 128)
                    sp = spsum.tile([128, ICH], F32, tag="sp")
                    nc.tensor.matmul(sp[:], kpT_sb[0][:, jsl], qpT_sb[0][:, icsl],
                                     start=True, stop=False)
                    nc.tensor.matmul(sp[:], kpT_sb[1][:, jsl], qpT_sb[1][:, icsl],
                                     start=False, stop=True)
                    ex = expp.tile([128, ICH], BF, tag="ex")
                    nc.scalar.activation(ex[:], sp[:], AF.Exp)
                    for it in range(4):
                        nc.tensor.matmul(otiles[it][:],
                                         ex[:, it * 128:(it + 1) * 128],
                                         vp_sb[j][:],
                                         start=(j == 0), stop=(j == NJ - 1),
                                         skip_group_check=True)
                for it in range(4):
                    rt = norm.tile([128, 1], F32, tag="rt")
                    nc.vector.reciprocal(rt[:], otiles[it][:, D:H1])
                    ob = obuf.tile([128, D], F32, tag="ob")
                    nc.vector.tensor_scalar_mul(ob[:], otiles[it][:, 0:D], rt[:])
                    r0 = (ic * 4 + it) * 128
                    nc.sync.dma_start(out[r0:r0 + 128, :], ob[:])

    nc.compile()
    return nc


class TileKernel:
    """Helper: TileContext + ExitStack as one context manager."""

    def __init__(self, nc, tile_mod):
        self.nc = nc
        self.tile_mod = tile_mod
        self.stack = ExitStack()

    def __enter__(self):
        tc = self.stack.enter_context(self.tile_mod.TileContext(self.nc))
        return tc, self.stack

    def __exit__(self, *exc):
        return self.stack.__exit__(*exc)


def _get_nc():
    if "nc" not in _CACHE:
        _CACHE["nc"] = _build()
    return _CACHE["nc"]


def kernel(q, k, v, Wq, bq, Wk, bk, Wv, bv):
    from concourse.bass_utils import run_bass_kernel_spmd

    q = np.asarray(q, dtype=np.float32)
    k = np.asarray(k, dtype=np.float32)
    v = np.asarray(v, dtype=np.float32)
    Wq = np.asarray(Wq, dtype=np.float32)
    bq = np.asarray(bq, dtype=np.float32)
    Wk = np.asarray(Wk, dtype=np.float32)
    bk = np.asarray(bk, dtype=np.float32)
    Wv = np.asarray(Wv, dtype=np.float32)
    bv = np.asarray(bv, dtype=np.float32)

    bf = ml_dtypes.bfloat16
    s = 1.0 / math.sqrt(D)

    WqT = (s * Wq.T).astype(bf)          # [d, e], softmax scale folded in
    WkT = Wk.T.astype(bf)                # [d, e]
    WvT = Wv.T.astype(bf)                # [d, h]
    bqs = (s * bq).reshape(D, 1).astype(np.float32)
    bk2 = bk.reshape(D, 1).astype(np.float32)
    bv1 = np.concatenate([bv, np.ones(1, np.float32)]).reshape(1, H1).astype(bf)

    shared = dict(WqT=WqT, WkT=WkT, WvT=WvT, bqs=bqs, bk=bk2, bv1=bv1)
    in_maps = []
    for core in range(8):
        b, half = divmod(core, 2)
        qs = slice(half * SQ, (half + 1) * SQ)
        in_maps.append(dict(
            qT=q[b, qs, :].T.astype(bf),
            kT=k[b].T.astype(bf),
            vT=v[b].T.astype(bf),
            **shared,
        ))

    global _last_in_maps
    _last_in_maps = in_maps

    nc = _get_nc()
    res = run_bass_kernel_spmd(nc, in_maps, core_ids=list(range(8)))

    full = np.empty((B, S, D), np.float32)
    for core in range(8):
        b, half = divmod(core, 2)
        full[b, half * SQ:(half + 1) * SQ, :] = res.results[core]["out"]
    return full


# revision 6
# speedup vs baseline: 1.0961x; 1.0961x over previous
"""Distributed attention kernel for Trainium2 (8 NeuronCores, SPMD).

Problem: B=4, S=4096, D=256 attention with QKV linear projections.
Sharding: core = 2*b + half -> batch b, query rows [half*2048, (half+1)*2048),
full K/V for that batch on every core (data-parallel batch + seq-parallel q).

Device-side layout trick: all sequence tensors are pre-transposed on the host
to [D, seq] so every matmul contraction dim lands on the partition axis and no
on-device transposes are needed. Scores are computed transposed
(scoresT[j, i] = kp . qp) so softmax exp is purely elementwise and the
AV matmul consumes exp tiles directly as the stationary operand.
Row-sums are obtained by appending a ones-column to vp (rank-1 matmul also
folds in the v-bias), so normalization is a cheap reciprocal+scale at the end.

All inputs are loaded in 1024-column chunks split across two DMA engines and
each projection chunk is a separate SBUF tile, so compute starts as soon as
the first chunks land instead of after the full 5 MB input load.
"""

import math
from contextlib import ExitStack

import numpy as np
import ml_dtypes

B = 4
S = 4096
D = 256
SQ = 2048  # query rows per core
NE = 2  # number of 128-row tiles covering D
ICH = 512  # i-chunk (query columns per scoresT matmul)
NIC = SQ // ICH  # 4
NJ = S // 128  # 32 j-tiles
H1 = D + 1  # vp width including the ones column
LCH = 1024  # DMA load chunk (columns)
PCH = 512  # projection chunk (columns)

_CACHE = {}
_last_in_maps = None


def _build():
    import concourse.bass as bass
    import concourse.tile as tile
    from concourse import bacc, mybir

    BF = mybir.dt.bfloat16
    F32 = mybir.dt.float32
    AF = mybir.ActivationFunctionType

    nc = bacc.Bacc("TRN2", target_bir_lowering=False, debug=False)

    qT = nc.declare_dram_parameter("qT", [D, SQ], BF, isOutput=False)
    kT = nc.declare_dram_parameter("kT", [D, S], BF, isOutput=False)
    vT = nc.declare_dram_parameter("vT", [D, S], BF, isOutput=False)
    WqT = nc.declare_dram_parameter("WqT", [D, D], BF, isOutput=False)
    WkT = nc.declare_dram_parameter("WkT", [D, D], BF, isOutput=False)
    WvT = nc.declare_dram_parameter("WvT", [D, D], BF, isOutput=False)
    bqs = nc.declare_dram_parameter("bqs", [D, 1], F32, isOutput=False)
    bk = nc.declare_dram_parameter("bk", [D, 1], F32, isOutput=False)
    bv1 = nc.declare_dram_parameter("bv1", [1, H1], BF, isOutput=False)
    out = nc.declare_dram_parameter("out", [SQ, D], F32, isOutput=True)

    with TileKernel(nc, tile) as (tc, ctx):
        const = ctx.enter_context(tc.tile_pool(name="const", bufs=1))
        inp = ctx.enter_context(tc.tile_pool(name="inp", bufs=1))
        persist = ctx.enter_context(tc.tile_pool(name="persist", bufs=1))

        # --- weights/biases/constants (gpsimd queue; tiny, must land first) ---
        def wload(dram, tag):
            ts = []
            for dt in range(NE):
                t = const.tile([128, D], BF, tag=f"{tag}{dt}", name=f"{tag}{dt}")
                nc.gpsimd.dma_start(t[:], dram[dt * 128:(dt + 1) * 128, :])
                ts.append(t)
            return ts

        wq_sb = wload(WqT, "wq")
        bqs_sb = []
        bk_sb = []
        for et in range(NE):
            t = const.tile([128, 1], F32, tag=f"bqs{et}", name=f"bqs{et}")
            nc.gpsimd.dma_start(t[:], bqs[et * 128:(et + 1) * 128, :])
            bqs_sb.append(t)
        wk_sb = wload(WkT, "wk")
        for et in range(NE):
            t = const.tile([128, 1], F32, tag=f"bk{et}", name=f"bk{et}")
            nc.gpsimd.dma_start(t[:], bk[et * 128:(et + 1) * 128, :])
            bk_sb.append(t)
        wv_sb = wload(WvT, "wv")
        bv1_sb = const.tile([1, H1], BF, tag="bv1")
        nc.gpsimd.dma_start(bv1_sb[:], bv1[:])
        ones1 = const.tile([1, 128], BF, tag="ones1")
        nc.vector.memset(ones1[:], 1.0)

        # --- chunked input loads (sync queue = HWDGE) ---
        # per (et, chunk) tiles of [128, LCH]
        def cload(dram, cols, tag):
            ts = [[None] * (cols // LCH) for _ in range(NE)]
            for c in range(cols // LCH):
                for dt in range(NE):
                    t = inp.tile([128, LCH], BF, tag=f"{tag}{dt}_{c}",
                                 name=f"{tag}{dt}_{c}")
                    nc.sync.dma_start(
                        t[:], dram[dt * 128:(dt + 1) * 128, c * LCH:(c + 1) * LCH])
                    ts[dt][c] = t
            return ts

        qT_sb = cload(qT, SQ, "qT")
        kT_sb = cload(kT, S, "kT")
        vT_sb = cload(vT, S, "vT")

        def in_ap(ts, col0, width):
            """AP into the chunked tiles for [col0, col0+width) per et."""
            c, off = divmod(col0, LCH)
            assert off + width <= LCH
            return [ts[dt][c][:, off:off + width] for dt in range(NE)]

        # --- projections (chunked tiles, emitted in attention-consumption order)
        qpT_sb = [[None] * (SQ // PCH) for _ in range(NE)]
        kpT_sb = [[None] * (S // PCH) for _ in range(NE)]
        vp_sb = [persist.tile([128, H1], BF, tag=f"vp{j}", name=f"vp{j}")
                 for j in range(NJ)]

        with tc.tile_pool(name="ppsum", bufs=3, space=bass.MemorySpace.PSUM) as ppsum, \
             tc.tile_pool(name="vpsum", bufs=3, space=bass.MemorySpace.PSUM) as vpsum:
            # qpT[e, i] = sum_d WqT_s[d, e] * qT[d, i]  (+ s*bq)
            for c in range(SQ // PCH):
                src = in_ap(qT_sb, c * PCH, PCH)
                for et in range(NE):
                    esl = slice(et * 128, (et + 1) * 128)
                    pp = ppsum.tile([128, PCH], F32, tag="pp")
                    nc.tensor.matmul(pp[:], wq_sb[0][:, esl], src[0],
                                     start=True, stop=False)
                    nc.tensor.matmul(pp[:], wq_sb[1][:, esl], src[1],
                                     start=False, stop=True)
                    t = persist.tile([128, PCH], BF, tag=f"qpT{et}_{c}",
                                     name=f"qpT{et}_{c}")
                    nc.vector.tensor_scalar_add(t[:], pp[:], bqs_sb[et][:])
                    qpT_sb[et][c] = t
            # kpT[e, j] = sum_d WkT[d, e] * kT[d, j]  (+ bk)
            for c in range(S // PCH):
                src = in_ap(kT_sb, c * PCH, PCH)
                for et in range(NE):
                    esl = slice(et * 128, (et + 1) * 128)
                    pp = ppsum.tile([128, PCH], F32, tag="pp")
                    nc.tensor.matmul(pp[:], wk_sb[0][:, esl], src[0],
                                     start=True, stop=False)
                    nc.tensor.matmul(pp[:], wk_sb[1][:, esl], src[1],
                                     start=False, stop=True)
                    t = persist.tile([128, PCH], BF, tag=f"kpT{et}_{c}",
                                     name=f"kpT{et}_{c}")
                    nc.vector.tensor_scalar_add(t[:], pp[:], bk_sb[et][:])
                    kpT_sb[et][c] = t
            # vp[j, h] = sum_d vT[d, j] * WvT[d, h] + bv[h]; col D is all-ones
            for j in range(NJ):
                src = in_ap(vT_sb, j * 128, 128)
                pv = vpsum.tile([128, H1], F32, tag="pv")
                nc.tensor.matmul(pv[:, 0:D], src[0], wv_sb[0][:],
                                 start=True, stop=False, skip_group_check=True)
                nc.tensor.matmul(pv[:, 0:D], src[1], wv_sb[1][:],
                                 start=False, stop=False, skip_group_check=True)
                nc.tensor.matmul(pv[:, 0:H1], ones1[:], bv1_sb[:],
                                 start=False, stop=True, skip_group_check=True)
                nc.scalar.activation(vp_sb[j][:], pv[:], AF.Identity)

        def kp_ap(j):
            """[128, 128] lhsT windows of kpT for j-tile j, per et."""
            c, off = divmod(j * 128, PCH)
            return [kpT_sb[et][c][:, off:off + 128] for et in range(NE)]

        # --- attention ---
        with tc.tile_pool(name="spsum", bufs=4, space=bass.MemorySpace.PSUM) as spsum, \
             tc.tile_pool(name="opsum", bufs=1, space=bass.MemorySpace.PSUM) as opsum, \
             tc.tile_pool(name="expp", bufs=4) as expp, \
             tc.tile_pool(name="norm", bufs=4) as norm, \
             tc.tile_pool(name="obuf", bufs=4) as obuf:
            for ic in range(NIC):
                qp_rhs = [qpT_sb[et][ic] for et in range(NE)]  # PCH == ICH
                otiles = [opsum.tile([128, H1], F32, tag=f"ot{it}",
                                     name=f"ot{ic}_{it}") for it in range(4)]
                for j in range(NJ):
                    kpw = kp_ap(j)
                    sp = spsum.tile([128, ICH], F32, tag="sp")
                    nc.tensor.matmul(sp[:], kpw[0], qp_rhs[0][:],
                                     start=True, stop=False)
                    nc.tensor.matmul(sp[:], kpw[1], qp_rhs[1][:],
                                     start=False, stop=True)
                    ex = expp.tile([128, ICH], BF, tag="ex")
                    nc.scalar.activation(ex[:], sp[:], AF.Exp)
                    for it in range(4):
                        nc.tensor.matmul(otiles[it][:],
                                         ex[:, it * 128:(it + 1) * 128],
                                         vp_sb[j][:],
                                         start=(j == 0), stop=(j == NJ - 1),
                                         skip_group_check=True)
                for it in range(4):
                    rt = norm.tile([128, 1], F32, tag="rt")
                    nc.vector.reciprocal(rt[:], otiles[it][:, D:H1])
                    ob = obuf.tile([128, D], F32, tag="ob")
                    nc.vector.tensor_scalar_mul(ob[:], otiles[it][:, 0:D], rt[:])
                    r0 = (ic * 4 + it) * 128
                    nc.gpsimd.dma_start(out[r0:r0 + 128, :], ob[:])

    nc.compile()
    return nc


class TileKernel:
    """Helper: TileContext + ExitStack as one context manager."""

    def __init__(self, nc, tile_mod):
        self.nc = nc
        self.tile_mod = tile_mod
        self.stack = ExitStack()

    def __enter__(self):
        tc = self.stack.enter_context(self.tile_mod.TileContext(self.nc))
        return tc, self.stack

    def __exit__(self, *exc):
        return self.stack.__exit__(*exc)


def _get_nc():
    if "nc" not in _CACHE:
        _CACHE["nc"] = _build()
    return _CACHE["nc"]


def kernel(q, k, v, Wq, bq, Wk, bk, Wv, bv):
    from concourse.bass_utils import run_bass_kernel_spmd

    q = np.asarray(q, dtype=np.float32)
    k = np.asarray(k, dtype=np.float32)
    v = np.asarray(v, dtype=np.float32)
    Wq = np.asarray(Wq, dtype=np.float32)
    bq = np.asarray(bq, dtype=np.float32)
    Wk = np.asarray(Wk, dtype=np.float32)
    bk = np.asarray(bk, dtype=np.float32)
    Wv = np.asarray(Wv, dtype=np.float32)
    bv = np.asarray(bv, dtype=np.float32)

    bf = ml_dtypes.bfloat16
    s = 1.0 / math.sqrt(D)

    WqT = (s * Wq.T).astype(bf)          # [d, e], softmax scale folded in
    WkT = Wk.T.astype(bf)                # [d, e]
    WvT = Wv.T.astype(bf)                # [d, h]
    bqs = (s * bq).reshape(D, 1).astype(np.float32)
    bk2 = bk.reshape(D, 1).astype(np.float32)
    bv1 = np.concatenate([bv, np.ones(1, np.float32)]).reshape(1, H1).astype(bf)

    shared = dict(WqT=WqT, WkT=WkT, WvT=WvT, bqs=bqs, bk=bk2, bv1=bv1)
    in_maps = []
    for core in range(8):
        b, half = divmod(core, 2)
        qs = slice(half * SQ, (half + 1) * SQ)
        in_maps.append(dict(
            qT=q[b, qs, :].T.astype(bf),
            kT=k[b].T.astype(bf),
            vT=v[b].T.astype(bf),
            **shared,
        ))

    global _last_in_maps
    _last_in_maps = in_maps

    nc = _get_nc()
    res = run_bass_kernel_spmd(nc, in_maps, core_ids=list(range(8)))

    full = np.empty((B, S, D), np.float32)
    for core in range(8):
        b, half = divmod(core, 2)
        full[b, half * SQ:(half + 1) * SQ, :] = res.results[core]["out"]
    return full


# revision 8
# speedup vs baseline: 1.3269x; 1.2105x over previous
"""Distributed attention kernel for Trainium2 (8 NeuronCores, SPMD).

Problem: B=4, S=4096, D=256 attention with QKV linear projections.
Sharding: core = 2*b + half -> batch b, query rows [half*2048, (half+1)*2048),
full K/V for that batch on every core (data-parallel batch + seq-parallel q).

Device-side layout trick: all sequence tensors are pre-transposed on the host
to [D, seq] so every matmul contraction dim lands on the partition axis and no
on-device transposes are needed. Scores are computed transposed
(scoresT[j, i] = kp . qp) so softmax exp is purely elementwise and the
AV matmul consumes exp tiles directly as the stationary operand.
Row-sums are obtained by appending a ones-column to vp (rank-1 matmul also
folds in the v-bias), so normalization is a cheap reciprocal+scale at the end.

All inputs are loaded in 1024-column chunks split across two DMA engines and
each projection chunk is a separate SBUF tile, so compute starts as soon as
the first chunks land instead of after the full 5 MB input load.
"""

import math
from contextlib import ExitStack

import numpy as np
import ml_dtypes

B = 4
S = 4096
D = 256
SQ = 2048  # query rows per core
NE = 2  # number of 128-row tiles covering D
ICH = 512  # i-chunk (query columns per scoresT matmul)
NIC = SQ // ICH  # 4
NJ = S // 128  # 32 j-tiles
H1 = D + 1  # vp width including the ones column
LCH = 1024  # DMA load chunk (columns)
PCH = 512  # projection chunk (columns)

_CACHE = {}
_last_in_maps = None


def _build():
    import concourse.bass as bass
    import concourse.tile as tile
    from concourse import bacc, mybir

    BF = mybir.dt.bfloat16
    F8 = mybir.dt.float8e4
    F32 = mybir.dt.float32
    AF = mybir.ActivationFunctionType

    nc = bacc.Bacc("TRN2", target_bir_lowering=False, debug=False)

    qT = nc.declare_dram_parameter("qT", [D, SQ], BF, isOutput=False)
    kT = nc.declare_dram_parameter("kT", [D, S], BF, isOutput=False)
    vT = nc.declare_dram_parameter("vT", [D, S], BF, isOutput=False)
    WqT = nc.declare_dram_parameter("WqT", [D, D], BF, isOutput=False)
    WkT = nc.declare_dram_parameter("WkT", [D, D], BF, isOutput=False)
    WvT = nc.declare_dram_parameter("WvT", [D, D], BF, isOutput=False)
    bqs = nc.declare_dram_parameter("bqs", [D, 1], F32, isOutput=False)
    bk = nc.declare_dram_parameter("bk", [D, 1], F32, isOutput=False)
    bv1 = nc.declare_dram_parameter("bv1", [1, H1], BF, isOutput=False)
    out = nc.declare_dram_parameter("out", [SQ, D], F32, isOutput=True)

    with TileKernel(nc, tile) as (tc, ctx):
        const = ctx.enter_context(tc.tile_pool(name="const", bufs=1))
        inp = ctx.enter_context(tc.tile_pool(name="inp", bufs=1))
        persist = ctx.enter_context(tc.tile_pool(name="persist", bufs=1))

        # --- weights/biases/constants (gpsimd queue; tiny, must land first) ---
        def wload(dram, tag):
            ts = []
            for dt in range(NE):
                t = const.tile([128, D], BF, tag=f"{tag}{dt}", name=f"{tag}{dt}")
                nc.gpsimd.dma_start(t[:], dram[dt * 128:(dt + 1) * 128, :])
                ts.append(t)
            return ts

        wq_sb = wload(WqT, "wq")
        bqs_sb = []
        bk_sb = []
        for et in range(NE):
            t = const.tile([128, 1], F32, tag=f"bqs{et}", name=f"bqs{et}")
            nc.gpsimd.dma_start(t[:], bqs[et * 128:(et + 1) * 128, :])
            bqs_sb.append(t)
        wk_sb = wload(WkT, "wk")
        for et in range(NE):
            t = const.tile([128, 1], F32, tag=f"bk{et}", name=f"bk{et}")
            nc.gpsimd.dma_start(t[:], bk[et * 128:(et + 1) * 128, :])
            bk_sb.append(t)
        wv_sb = []
        for dt in range(NE):
            t = const.tile([128, H1], BF, tag=f"wv{dt}", name=f"wv{dt}")
            nc.vector.memset(t[:, D:H1], 0.0)
            nc.gpsimd.dma_start(t[:, 0:D], WvT[dt * 128:(dt + 1) * 128, :])
            wv_sb.append(t)
        bv1_sb = const.tile([1, H1], BF, tag="bv1")
        nc.gpsimd.dma_start(bv1_sb[:], bv1[:])
        ones1 = const.tile([1, 128], BF, tag="ones1")
        nc.vector.memset(ones1[:], 1.0)

        # --- chunked input loads (sync queue = HWDGE) ---
        # per (et, chunk) tiles of [128, LCH]
        def cload(dram, cols, tag):
            ts = [[None] * (cols // LCH) for _ in range(NE)]
            for c in range(cols // LCH):
                for dt in range(NE):
                    t = inp.tile([128, LCH], BF, tag=f"{tag}{dt}_{c}",
                                 name=f"{tag}{dt}_{c}")
                    nc.sync.dma_start(
                        t[:], dram[dt * 128:(dt + 1) * 128, c * LCH:(c + 1) * LCH])
                    ts[dt][c] = t
            return ts

        qT_sb = cload(qT, SQ, "qT")
        kT_sb = cload(kT, S, "kT")
        vT_sb = cload(vT, S, "vT")

        def in_ap(ts, col0, width):
            """AP into the chunked tiles for [col0, col0+width) per et."""
            c, off = divmod(col0, LCH)
            assert off + width <= LCH
            return [ts[dt][c][:, off:off + width] for dt in range(NE)]

        # --- projections (chunked tiles, emitted in attention-consumption order)
        # qpT/kpT are stored as fp8e4 in K-group-interleaved layout [128, 2, cols]
        # so the scores matmul runs one DoubleRow matmul with K=256.
        qpT_sb = [persist.tile([128, NE, PCH], F8, tag=f"qpT{c}", name=f"qpT{c}")
                  for c in range(SQ // PCH)]
        kpT_sb = [persist.tile([128, NE, PCH], F8, tag=f"kpT{c}", name=f"kpT{c}")
                  for c in range(S // PCH)]
        vp_sb = [persist.tile([128, H1], BF, tag=f"vp{j}", name=f"vp{j}")
                 for j in range(NJ)]

        with tc.tile_pool(name="ppsum", bufs=3, space=bass.MemorySpace.PSUM) as ppsum, \
             tc.tile_pool(name="vpsum", bufs=3, space=bass.MemorySpace.PSUM) as vpsum:
            # qpT[e, i] = sum_d WqT_s[d, e] * qT[d, i]  (+ s*bq)
            for c in range(SQ // PCH):
                src = in_ap(qT_sb, c * PCH, PCH)
                for et in range(NE):
                    esl = slice(et * 128, (et + 1) * 128)
                    pp = ppsum.tile([128, PCH], F32, tag="pp")
                    nc.tensor.matmul(pp[:], wq_sb[0][:, esl], src[0],
                                     start=True, stop=False)
                    nc.tensor.matmul(pp[:], wq_sb[1][:, esl], src[1],
                                     start=False, stop=True)
                    nc.vector.tensor_scalar_add(qpT_sb[c][:, et, :], pp[:],
                                                bqs_sb[et][:])
            # kpT[e, j] = sum_d WkT[d, e] * kT[d, j]  (+ bk)
            for c in range(S // PCH):
                src = in_ap(kT_sb, c * PCH, PCH)
                for et in range(NE):
                    esl = slice(et * 128, (et + 1) * 128)
                    pp = ppsum.tile([128, PCH], F32, tag="pp")
                    nc.tensor.matmul(pp[:], wk_sb[0][:, esl], src[0],
                                     start=True, stop=False)
                    nc.tensor.matmul(pp[:], wk_sb[1][:, esl], src[1],
                                     start=False, stop=True)
                    nc.vector.tensor_scalar_add(kpT_sb[c][:, et, :], pp[:],
                                                bk_sb[et][:])
            # vp[j, h] = sum_d vT[d, j] * WvT[d, h] + bv[h]; col D is all-ones
            for j in range(NJ):
                src = in_ap(vT_sb, j * 128, 128)
                pv = vpsum.tile([128, H1], F32, tag="pv")
                nc.tensor.matmul(pv[:], src[0], wv_sb[0][:],
                                 start=True, stop=False, skip_group_check=True)
                nc.tensor.matmul(pv[:], src[1], wv_sb[1][:],
                                 start=False, stop=False, skip_group_check=True)
                nc.tensor.matmul(pv[:], ones1[:], bv1_sb[:],
                                 start=False, stop=True, skip_group_check=True)
                nc.scalar.activation(vp_sb[j][:], pv[:], AF.Identity)

        def kp_ap(j):
            """[128, 2, 128] DoubleRow lhsT window of kpT for j-tile j."""
            c, off = divmod(j * 128, PCH)
            return kpT_sb[c][:, :, off:off + 128]

        # --- attention ---
        with tc.tile_pool(name="spsum", bufs=4, space=bass.MemorySpace.PSUM) as spsum, \
             tc.tile_pool(name="opsum", bufs=1, space=bass.MemorySpace.PSUM) as opsum, \
             tc.tile_pool(name="expp", bufs=4) as expp, \
             tc.tile_pool(name="norm", bufs=4) as norm, \
             tc.tile_pool(name="obuf", bufs=4) as obuf:
            for ic in range(NIC):
                qp_rhs = qpT_sb[ic]  # PCH == ICH
                otiles = [opsum.tile([128, H1], F32, tag=f"ot{it}",
                                     name=f"ot{ic}_{it}") for it in range(4)]
                for j in range(NJ):
                    kpw = kp_ap(j)
                    sp = spsum.tile([128, ICH], F32, tag="sp")
                    nc.tensor.matmul(sp[:], kpw, qp_rhs[:],
                                     start=True, stop=True,
                                     perf_mode=mybir.MatmulPerfMode.DoubleRow)
                    ex = expp.tile([128, ICH], BF, tag="ex")
                    nc.scalar.activation(ex[:], sp[:], AF.Exp)
                    for it in range(4):
                        nc.tensor.matmul(otiles[it][:],
                                         ex[:, it * 128:(it + 1) * 128],
                                         vp_sb[j][:],
                                         start=(j == 0), stop=(j == NJ - 1),
                                         skip_group_check=True)
                for it in range(4):
                    rt = norm.tile([128, 1], F32, tag="rt")
                    nc.vector.reciprocal(rt[:], otiles[it][:, D:H1])
                    ob = obuf.tile([128, D], F32, tag="ob")
                    nc.vector.tensor_scalar_mul(ob[:], otiles[it][:, 0:D], rt[:])
                    r0 = (ic * 4 + it) * 128
                    nc.gpsimd.dma_start(out[r0:r0 + 128, :], ob[:])

    nc.compile()
    return nc


class TileKernel:
    """Helper: TileContext + ExitStack as one context manager."""

    def __init__(self, nc, tile_mod):
        self.nc = nc
        self.tile_mod = tile_mod
        self.stack = ExitStack()

    def __enter__(self):
        tc = self.stack.enter_context(self.tile_mod.TileContext(self.nc))
        return tc, self.stack

    def __exit__(self, *exc):
        return self.stack.__exit__(*exc)


def _get_nc():
    if "nc" not in _CACHE:
        _CACHE["nc"] = _build()
    return _CACHE["nc"]


def kernel(q, k, v, Wq, bq, Wk, bk, Wv, bv):
    from concourse.bass_utils import run_bass_kernel_spmd

    q = np.asarray(q, dtype=np.float32)
    k = np.asarray(k, dtype=np.float32)
    v = np.asarray(v, dtype=np.float32)
    Wq = np.asarray(Wq, dtype=np.float32)
    bq = np.asarray(bq, dtype=np.float32)
    Wk = np.asarray(Wk, dtype=np.float32)
    bk = np.asarray(bk, dtype=np.float32)
    Wv = np.asarray(Wv, dtype=np.float32)
    bv = np.asarray(bv, dtype=np.float32)

    bf = ml_dtypes.bfloat16
    s = 1.0 / math.sqrt(D)

    WqT = (s * Wq.T).astype(bf)          # [d, e], softmax scale folded in
    WkT = Wk.T.astype(bf)                # [d, e]
    WvT = Wv.T.astype(bf)                # [d, h]
    bqs = (s * bq).reshape(D, 1).astype(np.float32)
    bk2 = bk.reshape(D, 1).astype(np.float32)
    bv1 = np.concatenate([bv, np.ones(1, np.float32)]).reshape(1, H1).astype(bf)

    shared = dict(WqT=WqT, WkT=WkT, WvT=WvT, bqs=bqs, bk=bk2, bv1=bv1)
    in_maps = []
    for core in range(8):
        b, half = divmod(core, 2)
        qs = slice(half * SQ, (half + 1) * SQ)
        in_maps.append(dict(
            qT=q[b, qs, :].T.astype(bf),
            kT=k[b].T.astype(bf),
            vT=v[b].T.astype(bf),
            **shared,
        ))

    global _last_in_maps
    _last_in_maps = in_maps

    nc = _get_nc()
    res = run_bass_kernel_spmd(nc, in_maps, core_ids=list(range(8)))

    full = np.empty((B, S, D), np.float32)
    for core in range(8):
        b, half = divmod(core, 2)
        full[b, half * SQ:(half + 1) * SQ, :] = res.results[core]["out"]
    return full
